# revision 5
# baseline (speedup 1.0000x reference)
"""Binary position embedding kernel for Trainium2, 8-core data-parallel.

out[t, :] = sum_b bit_b(x[t]) * weight[b, :]  ==  bits(x) @ weight

Sharding: x flat [32768] -> 8 shards of 4096 tokens; weight replicated.
Each core computes its [4096, 1024] f32 output slice:
  - broadcast-DMA x shard onto 26 partitions
  - bitsT[b, t] = (x[t] >> (b % 13)) & 1   (DVE shift+and, gpsimd cast to bf16)
  - weight split into bf16 hi/lo rows stacked as [26, 1024] (keeps fp32-level
    accuracy with bf16 matmuls: out = bits@hi + bits@lo, fp32 PSUM accumulate)
  - per 128-token tile: 2 matmuls (N=512 each) -> PSUM, DVE/ACT copy -> SBUF,
    HWDGE DMA to HBM
"""

import numpy as np

import concourse.bass as bass
import concourse.mybir as mybir
from concourse.bass_utils import run_bass_kernel_spmd
from concourse.tile import TileContext

N_CORES = 8
B, S, D = 4, 8192, 1024
NB = 13           # bits per position
# Compute-engine APs need partition base % 32 == 0, so the hi rows live at
# partitions 0-12 and the lo rows at 32-44; the 13-31 gap has zeroed weight
# rows (so garbage bits there contribute nothing to the matmul).
LO_BASE = 32
KP = LO_BASE + NB          # 45 partitions in the stacked layout
TOK = (B * S) // N_CORES   # 4096 tokens per core
TILE = 128
NT = TOK // TILE           # 32 tiles
CHUNK = 1024               # tokens per bits-compute chunk

# test-harness knobs (ignored by plain kernel() use)
TRACE = False
LAST_RESULTS = None

_wsplit_counter = [0]


def _split_multi_waits(nc):
    """This env's walrus allows only one sync-wait per instruction. Hoist
    extra semaphore waits onto single-wait NoOps inserted just before the
    instruction on the same engine stream (same per-engine program order,
    identical blocking semantics)."""
    import bass_rust

    n_split = 0
    for f in nc.m.functions:
        for bb in f.blocks:
            insts = bb.instructions
            i = 0
            while i < len(insts):
                ins = insts[i]
                si = ins.sync_info
                if si is not None:
                    waits = list(si.on_wait)
                    sem_waits = [w for w in waits if w.sync_type == "semaphore"]
                    other = [w for w in waits if w.sync_type != "semaphore"]
                    keep = 1 if not other else 0
                    if len(waits) > 1 and len(sem_waits) > keep:
                        hoist = sem_waits[: len(sem_waits) - keep]
                        kept = sem_waits[len(sem_waits) - keep:]
                        si.on_wait = other + kept
                        for w in hoist:
                            noop = mybir.InstNoOp(
                                name=f"wsplit-{_wsplit_counter[0]}", ins=[], outs=[]
                            )
                            _wsplit_counter[0] += 1
                            noop.engine = ins.engine
                            noop.sync_info = bass_rust.SyncInfo(
                                on_wait=[w], on_update=[]
                            )
                            insts.insert(i, noop)
                            i += 1
                            n_split += 1
                i += 1
    return n_split


def _build():
    f32, bf16, i32 = mybir.dt.float32, mybir.dt.bfloat16, mybir.dt.int32
    op = mybir.AluOpType

    nc = bass.Bass()
    x = nc.declare_dram_parameter("x", [TOK], i32, isOutput=False)
    w = nc.declare_dram_parameter("weight", [NB, D], f32, isOutput=False)
    sh = nc.declare_dram_parameter("shifts", [KP, 1], i32, isOutput=False)
    out = nc.declare_dram_parameter("out", [TOK, D], f32, isOutput=True)

    with TileContext(nc) as tc:
        with (
            tc.tile_pool(name="const", bufs=1) as cpool,
            tc.tile_pool(name="outp", bufs=8) as opool,
            tc.tile_pool(name="psum", bufs=4, space="PSUM") as ppool,
        ):
            xb = cpool.tile([KP, TOK], i32)
            shs = cpool.tile([KP, 1], i32)
            w2 = cpool.tile([KP, D], f32)
            wstack = cpool.tile([KP, D], bf16)
            hi32 = cpool.tile([KP, D], f32)
            lo32 = cpool.tile([KP, D], f32)
            bits_i = cpool.tile([KP, TOK], i32)
            bitsT = cpool.tile([KP, TOK], bf16)

            nc.sync.dma_start(shs[:], sh[:])
            nc.gpsimd.dma_start(xb[:], x[None, :].broadcast_to([KP, TOK]))
            nc.sync.dma_start(w2[0:NB, :], w[:])
            nc.sync.dma_start(w2[LO_BASE:KP, :], w[:])

            # weight hi/lo split on gpsimd (keeps DVE free for PSUM copies).
            # All ops stay within their own partitions (no cross-lane moves).
            nc.gpsimd.memset(wstack[:], 0.0)                   # zero the gap rows
            nc.gpsimd.tensor_copy(wstack[0:NB, :], w2[0:NB, :])  # hi rows
            # hi recomputed at the lo partitions, then lo = w - hi there
            nc.gpsimd.tensor_copy(wstack[LO_BASE:KP, :], w2[LO_BASE:KP, :])
            nc.gpsimd.tensor_copy(hi32[LO_BASE:KP, :], wstack[LO_BASE:KP, :])
            nc.gpsimd.tensor_tensor(
                lo32[LO_BASE:KP, :], w2[LO_BASE:KP, :], hi32[LO_BASE:KP, :],
                op.subtract,
            )
            nc.gpsimd.tensor_copy(wstack[LO_BASE:KP, :], lo32[LO_BASE:KP, :])

            # bitsT in chunks so matmuls can start early
            for c in range(TOK // CHUNK):
                sl = slice(c * CHUNK, (c + 1) * CHUNK)
                nc.vector.tensor_scalar(
                    bits_i[:, sl], xb[:, sl], shs[:], 1,
                    op.logical_shift_right, op.bitwise_and,
                )
                nc.gpsimd.tensor_copy(bitsT[:, sl], bits_i[:, sl])

            for t in range(NT):
                ts = slice(t * TILE, (t + 1) * TILE)
                pt0 = ppool.tile([TILE, 512], f32)
                pt1 = ppool.tile([TILE, 512], f32)
                nc.tensor.matmul(
                    pt0[:], bitsT[:, ts], wstack[:, 0:512], start=True, stop=True
                )
                nc.tensor.matmul(
                    pt1[:], bitsT[:, ts], wstack[:, 512:D], start=True, stop=True
                )
                ot = opool.tile([TILE, D], f32)
                nc.vector.tensor_copy(ot[:, 0:512], pt0[:])
                nc.scalar.copy(ot[:, 512:D], pt1[:])
                nc.sync.dma_start(out[ts, :], ot[:])

    _split_multi_waits(nc)
    return nc


_nc_cache = None


def kernel(x, weight):
    global _nc_cache, LAST_RESULTS
    if _nc_cache is None:
        _nc_cache = _build()
    nc = _nc_cache

    xf = np.ascontiguousarray(np.asarray(x, dtype=np.int32).reshape(-1))
    wf = np.ascontiguousarray(np.asarray(weight, dtype=np.float32))
    shifts = (np.arange(KP, dtype=np.int32) % LO_BASE).reshape(KP, 1)
    shifts = np.minimum(shifts, NB - 1).astype(np.int32)  # gap rows: harmless
    in_maps = [
        {
            "x": np.ascontiguousarray(xf[c * TOK : (c + 1) * TOK]),
            "weight": wf,
            "shifts": shifts,
        }
        for c in range(N_CORES)
    ]
    res = run_bass_kernel_spmd(
        nc, in_maps, list(range(N_CORES)), trace=TRACE
    )
    LAST_RESULTS = res
    out = np.concatenate([r["out"] for r in res.results], axis=0)
    return out.reshape(B, S, D)


# revision 7
# speedup vs baseline: 1.2030x; 1.2030x over previous
"""Binary position embedding kernel for Trainium2, 8-core data-parallel.

out[t, :] = sum_b bit_b(x[t]) * weight[b, :]  ==  bits(x) @ weight

Sharding: x flat [32768] -> 8 shards of 4096 tokens; weight replicated.

Per-core plan (4096 tokens -> [4096, 1024] f32 = 16 MiB output, memory-bound):
  - x broadcast to all 128 SBUF partitions (HWDGE step-0 DMA).
  - Partition layout: 4 row groups of 32 partitions. In group g, rows
    32g+0..12 are "hi" bit rows, 32g+13..25 "lo" bit rows, 26..31 zero gap.
    bits[p, t] = (x[t] & mask[p]) != 0 computed in bf16 by two DVE ops
    (bitwise ops can't cast; the comparison op can).
  - weight split into bf16 hi/lo rows (w = hi + lo to ~16 mantissa bits) and
    replicated into the same 4-group layout; gap rows zero so garbage bits
    there contribute nothing.
  - Matmul: per 512-token supertile, 4 row groups run concurrently via
    tile_position=(32g, 0), each contracting K=26 over its own token tile,
    2 N-halves of 512 -> 8 PSUM banks.
  - PSUM -> SBUF copies split across DVE and ACT, then one 2 MiB HWDGE DMA
    per supertile to HBM (rows are 4 KiB contiguous).
"""

import numpy as np

import concourse.bass as bass
import concourse.mybir as mybir
from concourse.bass_utils import run_bass_kernel_spmd
from concourse.tile import TileContext

N_CORES = 8
B, S, D = 4, 8192, 1024
NB = 13                    # bits per position
GK = 2 * NB                # contraction rows per group (hi+lo)
NG = 4                     # row groups packed into the PE array
TOK = (B * S) // N_CORES   # 4096 tokens per core
TILE = 128
ST = NG * TILE             # 512 tokens per supertile
NST = TOK // ST            # 8 supertiles
CHUNK = 1024               # tokens per bits-compute chunk

TRACE = False
LAST_RESULTS = None

_wsplit_counter = [0]


def _split_multi_waits(nc):
    """This env's walrus allows only one sync-wait per instruction. Hoist
    extra semaphore waits onto single-wait NoOps inserted just before the
    instruction on the same engine stream (same per-engine program order,
    identical blocking semantics)."""
    import bass_rust

    n_split = 0
    for f in nc.m.functions:
        for bb in f.blocks:
            insts = bb.instructions
            i = 0
            while i < len(insts):
                ins = insts[i]
                si = ins.sync_info
                if si is not None:
                    waits = list(si.on_wait)
                    sem_waits = [w for w in waits if w.sync_type == "semaphore"]
                    other = [w for w in waits if w.sync_type != "semaphore"]
                    keep = 1 if not other else 0
                    if len(waits) > 1 and len(sem_waits) > keep:
                        hoist = sem_waits[: len(sem_waits) - keep]
                        kept = sem_waits[len(sem_waits) - keep:]
                        si.on_wait = other + kept
                        for w in hoist:
                            noop = mybir.InstNoOp(
                                name=f"wsplit-{_wsplit_counter[0]}", ins=[], outs=[]
                            )
                            _wsplit_counter[0] += 1
                            noop.engine = ins.engine
                            noop.sync_info = bass_rust.SyncInfo(
                                on_wait=[w], on_update=[]
                            )
                            insts.insert(i, noop)
                            i += 1
                            n_split += 1
                i += 1
    return n_split


def _build():
    f32, bf16, i32 = mybir.dt.float32, mybir.dt.bfloat16, mybir.dt.int32
    op = mybir.AluOpType

    nc = bass.Bass()
    x = nc.declare_dram_parameter("x", [TOK], i32, isOutput=False)
    w = nc.declare_dram_parameter("weight", [NB, D], f32, isOutput=False)
    mk = nc.declare_dram_parameter("mask", [128, 1], i32, isOutput=False)
    isl = nc.declare_dram_parameter("is_lo", [128, 1], f32, isOutput=False)
    out = nc.declare_dram_parameter("out", [TOK, D], f32, isOutput=True)

    with TileContext(nc) as tc:
        with (
            tc.tile_pool(name="const", bufs=1) as cpool,
            tc.tile_pool(name="outp", bufs=4) as opool,
            tc.tile_pool(name="psum", bufs=1, space="PSUM") as ppool,
        ):
            xb = cpool.tile([128, TOK], i32)
            mks = cpool.tile([128, 1], i32)
            isls = cpool.tile([128, 1], f32)
            w2 = cpool.tile([128, D], f32)
            wstack = cpool.tile([128, D], bf16)
            hi32 = cpool.tile([128, D], f32)
            tmp32 = cpool.tile([128, D], f32)
            bits_i = cpool.tile([128, TOK], i32)
            bitsT = cpool.tile([128, TOK], bf16)

            nc.sync.dma_start(mks[:], mk[:])
            nc.sync.dma_start(isls[:], isl[:])
            # weight replicated into hi and lo row slots of each group
            nc.gpsimd.memset(w2[:], 0.0)
            for g in range(NG):
                nc.sync.dma_start(w2[32 * g : 32 * g + NB, :], w[:])
                nc.sync.dma_start(w2[32 * g + NB : 32 * g + GK, :], w[:])
            # x broadcast to all partitions, in chunks so bits start early
            for c in range(TOK // CHUNK):
                sl = slice(c * CHUNK, (c + 1) * CHUNK)
                nc.sync.dma_start(
                    xb[:, sl], x[None, sl].broadcast_to([128, CHUNK])
                )

            # hi/lo weight prep, whole-tile ops only (no partition-offset
            # writes): wstack = bf16(w2 - is_lo * f32(bf16(w2)))
            nc.vector.tensor_copy(wstack[:], w2[:])        # bf16 round (hi)
            nc.vector.tensor_copy(hi32[:], wstack[:])      # back to f32
            nc.vector.tensor_scalar(
                tmp32[:], hi32[:], isls[:], None, op.mult
            )                                              # is_lo * hi
            nc.vector.tensor_tensor(tmp32[:], w2[:], tmp32[:], op.subtract)
            nc.vector.tensor_copy(wstack[:], tmp32[:])     # final bf16

            # bits: (x & mask) != 0  -> bf16  (two DVE ops per chunk)
            for c in range(TOK // CHUNK):
                sl = slice(c * CHUNK, (c + 1) * CHUNK)
                nc.vector.tensor_scalar(
                    bits_i[:, sl], xb[:, sl], mks[:], None, op.bitwise_and
                )
                nc.vector.tensor_scalar(
                    bitsT[:, sl], bits_i[:, sl], 0, None, op.not_equal
                )

            for s in range(NST):
                ob = opool.tile([TILE, NG * D], f32)
                for g in range(NG):
                    t0 = (s * NG + g) * TILE
                    for h in range(2):
                        pt = ppool.tile([TILE, 512], f32, tag=f"p{g}{h}")
                        nc.tensor.matmul(
                            pt[:],
                            bitsT[32 * g : 32 * g + GK, t0 : t0 + TILE],
                            wstack[32 * g : 32 * g + GK, 512 * h : 512 * (h + 1)],
                            start=True,
                            stop=True,
                            tile_position=(32 * g, 0),
                        )
                        dst = ob[:, g * D + 512 * h : g * D + 512 * (h + 1)]
                        if (g + h) % 2 == 0:
                            nc.vector.tensor_copy(dst, pt[:])
                        else:
                            nc.scalar.copy(dst, pt[:])
                # one 2 MiB DMA for the whole supertile
                dram_view = out[s * ST : (s + 1) * ST, :].rearrange(
                    "(g p) d -> p g d", p=TILE
                )
                nc.sync.dma_start(dram_view, ob[:].rearrange("p (g d) -> p g d", g=NG))

    _split_multi_waits(nc)
    return nc


_nc_cache = None


def kernel(x, weight):
    global _nc_cache, LAST_RESULTS
    if _nc_cache is None:
        _nc_cache = _build()
    nc = _nc_cache

    xf = np.ascontiguousarray(np.asarray(x, dtype=np.int32).reshape(-1))
    wf = np.ascontiguousarray(np.asarray(weight, dtype=np.float32))
    mask = np.zeros((128, 1), np.int32)
    is_lo = np.zeros((128, 1), np.float32)
    for p in range(128):
        r = p % 32
        if r < GK:
            mask[p, 0] = 1 << (r % NB)
            is_lo[p, 0] = 1.0 if r >= NB else 0.0
    in_maps = [
        {
            "x": np.ascontiguousarray(xf[c * TOK : (c + 1) * TOK]),
            "weight": wf,
            "mask": mask,
            "is_lo": is_lo,
        }
        for c in range(N_CORES)
    ]
    res = run_bass_kernel_spmd(nc, in_maps, list(range(N_CORES)), trace=TRACE)
    LAST_RESULTS = res
    out = np.concatenate([r["out"] for r in res.results], axis=0)
    return out.reshape(B, S, D)


# revision 9
# speedup vs baseline: 1.2426x; 1.0329x over previous
"""Binary position embedding kernel for Trainium2, 8-core data-parallel.

out[t, :] = sum_b bit_b(x[t]) * weight[b, :]  ==  bits(x) @ weight

Sharding: x flat [32768] -> 8 shards of 4096 tokens; weight replicated.

Per-core plan (4096 tokens -> [4096, 1024] f32 = 16 MiB output; the kernel is
bound by writing that at ~360 GB/s ~= 47 us, so everything else hides under
the output-DMA stream and the prologue is kept as short as possible):
  - x (as int16, values < 8192) broadcast to all 128 SBUF partitions.
  - Partition layout: 4 row groups of 32. In group g, rows 32g+0..12 are "hi"
    bit rows, 32g+13..25 "lo" bit rows, 26..31 zero gap.
    bits[p, t] = (x[t] & mask[p]) != 0 in bf16 via two DVE ops (bitwise ops
    can't dtype-cast; comparison ops can).
  - weight split into bf16 hi/lo rows (w = hi + lo, ~16 mantissa bits) on the
    scalar engine, replicated into the 4-group layout; gap rows zero so
    garbage bits there contribute nothing.
  - Matmul: per 512-token supertile, 4 row groups run concurrently via
    tile_position=(32g, 0), each contracting K=26 over its own token tile,
    2 N-halves of 512 -> 8 PSUM banks.
  - PSUM -> SBUF copies split across DVE and ACT, then one 2 MiB HWDGE DMA
    per supertile (issued from SP, which does nothing else) to HBM.
"""

import numpy as np

import concourse.bass as bass
import concourse.mybir as mybir
from concourse.bass_utils import run_bass_kernel_spmd
from concourse.tile import TileContext

N_CORES = 8
B, S, D = 4, 8192, 1024
NB = 13                    # bits per position
GK = 2 * NB                # contraction rows per group (hi+lo)
NG = 4                     # row groups packed into the PE array
TOK = (B * S) // N_CORES   # 4096 tokens per core
TILE = 128
ST = NG * TILE             # 512 tokens per supertile
NST = TOK // ST            # 8 supertiles
# bits chunk boundaries (tokens): small first chunk so supertile 0 starts early
CHUNKS = [512, 512, 1024, 2048]

TRACE = False
LAST_RESULTS = None

_wsplit_counter = [0]


def _split_multi_waits(nc):
    """This env's walrus allows only one sync-wait per instruction. Hoist
    extra semaphore waits onto single-wait NoOps inserted just before the
    instruction on the same engine stream (same per-engine program order,
    identical blocking semantics)."""
    import bass_rust

    n_split = 0
    for f in nc.m.functions:
        for bb in f.blocks:
            insts = bb.instructions
            i = 0
            while i < len(insts):
                ins = insts[i]
                si = ins.sync_info
                if si is not None:
                    waits = list(si.on_wait)
                    sem_waits = [w for w in waits if w.sync_type == "semaphore"]
                    other = [w for w in waits if w.sync_type != "semaphore"]
                    keep = 1 if not other else 0
                    if len(waits) > 1 and len(sem_waits) > keep:
                        hoist = sem_waits[: len(sem_waits) - keep]
                        kept = sem_waits[len(sem_waits) - keep:]
                        si.on_wait = other + kept
                        for w in hoist:
                            noop = mybir.InstNoOp(
                                name=f"wsplit-{_wsplit_counter[0]}", ins=[], outs=[]
                            )
                            _wsplit_counter[0] += 1
                            noop.engine = ins.engine
                            noop.sync_info = bass_rust.SyncInfo(
                                on_wait=[w], on_update=[]
                            )
                            insts.insert(i, noop)
                            i += 1
                            n_split += 1
                i += 1
    return n_split


def _build():
    f32, bf16 = mybir.dt.float32, mybir.dt.bfloat16
    i16 = mybir.dt.int16
    op = mybir.AluOpType

    nc = bass.Bass()
    x = nc.declare_dram_parameter("x", [TOK], i16, isOutput=False)
    w = nc.declare_dram_parameter("weight", [NB, D], f32, isOutput=False)
    mk = nc.declare_dram_parameter("mask", [128, 1], i16, isOutput=False)
    isl = nc.declare_dram_parameter("is_lo", [128, 1], f32, isOutput=False)
    out = nc.declare_dram_parameter("out", [TOK, D], f32, isOutput=True)

    with TileContext(nc) as tc:
        with (
            tc.tile_pool(name="const", bufs=1) as cpool,
            tc.tile_pool(name="outp", bufs=4) as opool,
            tc.tile_pool(name="psum", bufs=1, space="PSUM") as ppool,
        ):
            xb = cpool.tile([128, TOK], i16)
            mks = cpool.tile([128, 1], i16)
            isls = cpool.tile([128, 1], f32)
            w2 = cpool.tile([128, D], f32)
            wstack = cpool.tile([128, D], bf16)
            hi32 = cpool.tile([128, D], f32)
            tmp32 = cpool.tile([128, D], f32)
            bits_i = cpool.tile([128, TOK], i16)
            bitsT = cpool.tile([128, TOK], bf16)

            # --- input DMAs: none on SP (SP is reserved for output DMAs) ---
            nc.scalar.dma_start(mks[:], mk[:])
            nc.scalar.dma_start(isls[:], isl[:])
            # weight replicated into hi+lo slots of all 4 groups
            # (plain single-partition-range DMAs; nested partition APs are
            # silently misinterpreted by the DMA lowering)
            nc.gpsimd.memset(w2[:], 0.0)  # gap rows stay zero
            for g in range(NG):
                eng = nc.scalar if g % 2 == 0 else nc.gpsimd
                eng.dma_start(w2[32 * g : 32 * g + NB, :], w[:])
                eng.dma_start(w2[32 * g + NB : 32 * g + GK, :], w[:])
            # x broadcast chunks (int16): early chunks small
            off = 0
            for cl in CHUNKS:
                sl = slice(off, off + cl)
                nc.scalar.dma_start(
                    xb[:, sl], x[None, sl].broadcast_to([128, cl])
                )
                off += cl

            # --- weight hi/lo prep on ACT (+1 DVE subtract), whole-tile ops:
            #     wstack = bf16(w2 - is_lo * f32(bf16(w2)))
            nc.scalar.copy(wstack[:], w2[:])               # bf16 round (hi)
            nc.scalar.copy(hi32[:], wstack[:])             # back to f32
            nc.scalar.mul(tmp32[:], hi32[:], isls[:])      # is_lo * hi
            nc.vector.tensor_tensor(tmp32[:], w2[:], tmp32[:], op.subtract)
            nc.scalar.copy(wstack[:], tmp32[:])            # final bf16

            # --- bits: (x & mask) != 0 -> bf16 (two DVE ops per chunk) ---
            off = 0
            for cl in CHUNKS:
                sl = slice(off, off + cl)
                nc.vector.tensor_scalar(
                    bits_i[:, sl], xb[:, sl], mks[:], None, op.bitwise_and
                )
                nc.vector.tensor_scalar(
                    bitsT[:, sl], bits_i[:, sl], 0, None, op.not_equal
                )
                off += cl

            # --- main loop: 8 supertiles of 512 tokens ---
            for s in range(NST):
                ob = opool.tile([TILE, NG * D], f32)
                for g in range(NG):
                    t0 = (s * NG + g) * TILE
                    for h in range(2):
                        pt = ppool.tile([TILE, 512], f32, tag=f"p{g}{h}")
                        nc.tensor.matmul(
                            pt[:],
                            bitsT[32 * g : 32 * g + GK, t0 : t0 + TILE],
                            wstack[32 * g : 32 * g + GK, 512 * h : 512 * (h + 1)],
                            start=True,
                            stop=True,
                            tile_position=(32 * g, 0),
                        )
                        dst = ob[:, g * D + 512 * h : g * D + 512 * (h + 1)]
                        if (g + h) % 2 == 0:
                            nc.vector.tensor_copy(dst, pt[:])
                        else:
                            nc.scalar.copy(dst, pt[:])
                # one 2 MiB DMA for the whole supertile (SP engine only)
                dram_view = out[s * ST : (s + 1) * ST, :].rearrange(
                    "(g p) d -> p g d", p=TILE
                )
                nc.sync.dma_start(
                    dram_view, ob[:].rearrange("p (g d) -> p g d", g=NG)
                )

    _split_multi_waits(nc)
    return nc


_nc_cache = None


def kernel(x, weight):
    global _nc_cache, LAST_RESULTS
    if _nc_cache is None:
        _nc_cache = _build()
    nc = _nc_cache

    # x values are < 8192 so they fit int16 exactly (halves broadcast bytes)
    xf = np.ascontiguousarray(
        np.asarray(x, dtype=np.int32).reshape(-1).astype(np.int16)
    )
    wf = np.ascontiguousarray(np.asarray(weight, dtype=np.float32))
    mask = np.zeros((128, 1), np.int16)
    is_lo = np.zeros((128, 1), np.float32)
    for p in range(128):
        r = p % 32
        if r < GK:
            mask[p, 0] = 1 << (r % NB)
            is_lo[p, 0] = 1.0 if r >= NB else 0.0
    in_maps = [
        {
            "x": np.ascontiguousarray(xf[c * TOK : (c + 1) * TOK]),
            "weight": wf,
            "mask": mask,
            "is_lo": is_lo,
        }
        for c in range(N_CORES)
    ]
    res = run_bass_kernel_spmd(nc, in_maps, list(range(N_CORES)), trace=TRACE)
    LAST_RESULTS = res
    out = np.concatenate([r["out"] for r in res.results], axis=0)
    return out.reshape(B, S, D)


# revision 10
# speedup vs baseline: 1.3405x; 1.0788x over previous
"""Binary position embedding kernel for Trainium2, 8-core data-parallel.

out[t, :] = sum_b bit_b(x[t]) * weight[b, :]  ==  bits(x) @ weight

Sharding: x flat [32768] -> 8 shards of 4096 tokens; weight replicated
(host also pre-splits it into bf16 hi/lo rows and replicates into the
4-row-group SBUF layout, a lossless re-encoding of the tiny table).

Per-core plan (4096 tokens -> [4096, 1024] f32 = 16 MiB output; the kernel is
bound by writing that at ~360 GB/s ~= 47 us, so everything else must hide
under the output-DMA stream and the prologue must be minimal):
  - ONE staged input blob [128, 5124] int16 per core, DMA'd in 2 chunks that
    are hoisted before the kernel's startup barrier so the transfers overlap
    the fixed ~7 us preamble. Blob columns per partition p (r = p % 32):
      [0:1024]    wstack row bitcast: bf16 hi row (r<13) / lo row (13<=r<26)
                  of weight, zeros for gap rows 26..31
      [1024]      bit mask: 1 << (r % 13), 0 for gap rows
      [1028:5124] x broadcast (int16; values < 8192 fit exactly)
  - bits[p, t] = (x[t] & mask[p]) != 0 in bf16 via two DVE ops per chunk
    (bitwise ops can't dtype-cast; the comparison op can).
  - Matmul: per 512-token supertile, 4 row groups run concurrently via
    tile_position=(32g, 0), each contracting K=26 (hi+lo) over its own token
    tile, 2 N-halves of 512 -> 8 PSUM banks.
  - PSUM -> SBUF copies split across DVE and ACT; one 2 MiB HWDGE DMA per
    supertile, alternating between the SP and ACT HWDGE rings so the
    per-DMA edges overlap.
"""

import numpy as np
import ml_dtypes

import concourse.bass as bass
import concourse.mybir as mybir
from concourse.bass_utils import run_bass_kernel_spmd
from concourse.tile import TileContext

N_CORES = 8
B, S, D = 4, 8192, 1024
NB = 13                    # bits per position
GK = 2 * NB                # contraction rows per group (hi+lo)
NG = 4                     # row groups packed into the PE array
TOK = (B * S) // N_CORES   # 4096 tokens per core
TILE = 128
ST = NG * TILE             # 512 tokens per supertile
NST = TOK // ST            # 8 supertiles

# blob layout (int16 columns per partition)
W_COLS = D                 # 1024: wstack row (bf16 bits)
MASK_COL = W_COLS          # 1 column
X_OFF = W_COLS + 4         # x starts here (pad to 4-col alignment)
BLOB_COLS = X_OFF + TOK    # 5124
# first input DMA covers wstack+mask+first x chunk; second the rest
CHUNK0 = 1024
CHUNKS = [CHUNK0, 1024, 1024, 1024]

TRACE = False
LAST_RESULTS = None

_wsplit_counter = [0]


def _split_multi_waits(nc):
    """This env's walrus allows only one sync-wait per instruction. Hoist
    extra semaphore waits onto single-wait NoOps inserted just before the
    instruction on the same engine stream (same per-engine program order,
    identical blocking semantics)."""
    import bass_rust

    n_split = 0
    for f in nc.m.functions:
        for bb in f.blocks:
            insts = bb.instructions
            i = 0
            while i < len(insts):
                ins = insts[i]
                si = ins.sync_info
                if si is not None:
                    waits = list(si.on_wait)
                    sem_waits = [w for w in waits if w.sync_type == "semaphore"]
                    other = [w for w in waits if w.sync_type != "semaphore"]
                    keep = 1 if not other else 0
                    if len(waits) > 1 and len(sem_waits) > keep:
                        hoist = sem_waits[: len(sem_waits) - keep]
                        kept = sem_waits[len(sem_waits) - keep:]
                        si.on_wait = other + kept
                        for w in hoist:
                            noop = mybir.InstNoOp(
                                name=f"wsplit-{_wsplit_counter[0]}", ins=[], outs=[]
                            )
                            _wsplit_counter[0] += 1
                            noop.engine = ins.engine
                            noop.sync_info = bass_rust.SyncInfo(
                                on_wait=[w], on_update=[]
                            )
                            insts.insert(i, noop)
                            i += 1
                            n_split += 1
                i += 1
    return n_split


def _hoist_to_preamble(nc, names):
    """Move the named (wait-free) instructions from the body block to the
    preamble block, before the startup barrier, so their DMA transfers
    overlap the fixed kernel-start overhead."""
    main_bb = nc.m.functions[0].blocks[0]
    moved = []
    for f in nc.m.functions:
        for bb in f.blocks:
            if bb is main_bb:
                continue
            insts = bb.instructions
            i = 0
            while i < len(insts):
                if insts[i].name in names:
                    moved.append(insts.pop(i))
                else:
                    i += 1
    # insert after the leading Call/RegisterMove run
    pos = 0
    mi = main_bb.instructions
    while pos < len(mi) and mi[pos].opcode in ("Call", "RegisterMove"):
        pos += 1
    for j, ins in enumerate(moved):
        mi.insert(pos + j, ins)
    return len(moved)


def _build():
    f32, bf16 = mybir.dt.float32, mybir.dt.bfloat16
    i16 = mybir.dt.int16
    op = mybir.AluOpType

    nc = bass.Bass()
    blob = nc.declare_dram_parameter("blob", [128, BLOB_COLS], i16, isOutput=False)
    out = nc.declare_dram_parameter("out", [TOK, D], f32, isOutput=True)

    hoist_names = []
    with TileContext(nc) as tc:
        with (
            tc.tile_pool(name="const", bufs=1) as cpool,
            tc.tile_pool(name="outp", bufs=4) as opool,
            tc.tile_pool(name="psum", bufs=1, space="PSUM") as ppool,
        ):
            sb = cpool.tile([128, BLOB_COLS], i16)
            bits_i = cpool.tile([128, TOK], i16)
            bitsT = cpool.tile([128, TOK], bf16)

            wstack = sb[:, 0:W_COLS].bitcast(bf16)
            mks = sb[:, MASK_COL : MASK_COL + 1]

            # input DMAs (hoisted to the preamble by name below)
            d0 = nc.scalar.dma_start(
                sb[:, 0 : X_OFF + CHUNK0], blob[:, 0 : X_OFF + CHUNK0]
            )
            d1 = nc.scalar.dma_start(
                sb[:, X_OFF + CHUNK0 :], blob[:, X_OFF + CHUNK0 :]
            )
            hoist_names = [d0.ins.name, d1.ins.name]

            # bits: (x & mask) != 0 -> bf16 (two DVE ops per chunk)
            off = 0
            for cl in CHUNKS:
                xsl = sb[:, X_OFF + off : X_OFF + off + cl]
                nc.vector.tensor_scalar(
                    bits_i[:, off : off + cl], xsl, mks, None, op.bitwise_and
                )
                nc.vector.tensor_scalar(
                    bitsT[:, off : off + cl],
                    bits_i[:, off : off + cl],
                    0,
                    None,
                    op.not_equal,
                )
                off += cl

            # main loop: 8 supertiles of 512 tokens
            for s in range(NST):
                ob = opool.tile([TILE, NG * D], f32)
                for g in range(NG):
                    t0 = (s * NG + g) * TILE
                    for h in range(2):
                        pt = ppool.tile([TILE, 512], f32, tag=f"p{g}{h}")
                        nc.tensor.matmul(
                            pt[:],
                            bitsT[32 * g : 32 * g + GK, t0 : t0 + TILE],
                            wstack[32 * g : 32 * g + GK, 512 * h : 512 * (h + 1)],
                            start=True,
                            stop=True,
                            tile_position=(32 * g, 0),
                        )
                        dst = ob[:, g * D + 512 * h : g * D + 512 * (h + 1)]
                        if (g + h) % 2 == 0:
                            nc.vector.tensor_copy(dst, pt[:])
                        else:
                            nc.scalar.copy(dst, pt[:])
                # one 2 MiB DMA per supertile; alternate HWDGE rings
                dram_view = out[s * ST : (s + 1) * ST, :].rearrange(
                    "(g p) d -> p g d", p=TILE
                )
                eng = nc.sync if s % 2 == 0 else nc.scalar
                eng.dma_start(
                    dram_view, ob[:].rearrange("p (g d) -> p g d", g=NG)
                )

    _hoist_to_preamble(nc, set(hoist_names))
    _split_multi_waits(nc)
    return nc


_nc_cache = None


def _make_blob(xf_shard, weight):
    """Host-staged per-core input blob [128, BLOB_COLS] int16."""
    blob = np.zeros((128, BLOB_COLS), np.int16)
    w = np.asarray(weight, dtype=np.float32)
    hi = w.astype(ml_dtypes.bfloat16)
    lo = (w - hi.astype(np.float32)).astype(ml_dtypes.bfloat16)
    hi16 = hi.view(np.int16)
    lo16 = lo.view(np.int16)
    for g in range(NG):
        blob[32 * g : 32 * g + NB, 0:W_COLS] = hi16
        blob[32 * g + NB : 32 * g + GK, 0:W_COLS] = lo16
    for p in range(128):
        r = p % 32
        if r < GK:
            blob[p, MASK_COL] = 1 << (r % NB)
    blob[:, X_OFF:] = xf_shard[None, :]
    return blob


def kernel(x, weight):
    global _nc_cache, LAST_RESULTS
    if _nc_cache is None:
        _nc_cache = _build()
    nc = _nc_cache

    # x values are < 8192 so they fit int16 exactly
    xf = np.asarray(x, dtype=np.int32).reshape(-1).astype(np.int16)
    in_maps = [
        {"blob": _make_blob(xf[c * TOK : (c + 1) * TOK], weight)}
        for c in range(N_CORES)
    ]
    res = run_bass_kernel_spmd(nc, in_maps, list(range(N_CORES)), trace=TRACE)
    LAST_RESULTS = res
    out = np.concatenate([r["out"] for r in res.results], axis=0)
    return out.reshape(B, S, D)


# revision 14
# speedup vs baseline: 1.5706x; 1.1716x over previous
"""Binary position embedding kernel for Trainium2, 8-core data-parallel.

out[t, :] = sum_b bit_b(x[t]) * weight[b, :]  ==  bits(x) @ weight

Sharding: x flat [32768] -> 8 shards of 4096 tokens; weight replicated
(host also pre-splits it into bf16 hi/lo rows and replicates into the
4-row-group SBUF layout, a lossless re-encoding of the tiny table).

Per-core plan (4096 tokens -> [4096, 1024] f32 = 16 MiB output; the kernel is
bound by writing that at ~360 GB/s ~= 47 us, so everything else must hide
under the output-DMA stream and the prologue must be minimal):
  - ONE staged input blob [128, 5124] int16 per core, DMA'd in 2 chunks that
    are hoisted before the kernel's startup barrier so the transfers overlap
    the fixed ~7 us preamble. Blob columns per partition p (r = p % 32):
      [0:1024]    wstack row bitcast: bf16 hi row (r<13) / lo row (13<=r<26)
                  of weight, zeros for gap rows 26..31
      [1024]      bit mask: 1 << (r % 13), 0 for gap rows
      [1028:5124] x broadcast (int16; values < 8192 fit exactly)
  - bits[p, t] = (x[t] & mask[p]) != 0 in bf16 via two DVE ops per chunk
    (bitwise ops can't dtype-cast; the comparison op can).
  - Matmul: per 512-token supertile, 4 row groups run concurrently via
    tile_position=(32g, 0), each contracting K=26 (hi+lo) over its own token
    tile, 2 N-halves of 512 -> 8 PSUM banks.
  - PSUM -> SBUF copies split across DVE and ACT; one 2 MiB HWDGE DMA per
    supertile, alternating between the SP and ACT HWDGE rings so the
    per-DMA edges overlap.
"""

import numpy as np
import ml_dtypes

import concourse.bass as bass
import concourse.mybir as mybir
from concourse.bass_utils import run_bass_kernel_spmd
from concourse.tile import TileContext
from concourse.vector_clock import ScopedClock


class _LeanTailTileContext(TileContext):
    """Standard tail emits drain -> barrier -> sem clears -> barrier. The
    final barrier only syncs engine-stream ends after the gpsimd-only sem
    clears; dropping it shaves the second EVSEM butterfly off the critical
    path. Re-execution stays safe: clears still run after the full barrier,
    and the next run's entry barrier resynchronizes engines."""

    def _drain_and_barrier(self, tick_clock, wait_clock):
        nc = self.nc
        drain_inst = nc.sync.drain()
        wait_clock.add_sem_waits(
            drain_inst.ins, ScopedClock({None: tick_clock.global_clock})
        )
        nc.all_engine_barrier()
        popped = nc._tile_sem_poison_stack.pop()
        assert popped is self._sem_poison
        nc.clear_and_free_semaphores(list(self.sems.allocated().values()))

N_CORES = 8
B, S, D = 4, 8192, 1024
NB = 13                    # bits per position
GK = 2 * NB                # contraction rows per group (hi+lo)
NG = 4                     # row groups packed into the PE array
TOK = (B * S) // N_CORES   # 4096 tokens per core
TILE = 128
ST = NG * TILE             # 512 tokens per supertile
NST = TOK // ST            # 8 supertiles

# blob layout (int16 columns per partition)
W_COLS = D                 # 1024: wstack row (bf16 bits)
MASK_COL = W_COLS          # 1 column
X_OFF = W_COLS + 4         # x starts here (pad to 4-col alignment)
BLOB_COLS = X_OFF + TOK    # 5124
# first input DMA covers wstack+mask+first x chunk; second the rest
CHUNK0 = 512
CHUNKS = [CHUNK0, 512, 1024, 2048]

TRACE = False
LAST_RESULTS = None

_wsplit_counter = [0]


def _split_multi_waits(nc):
    """This env's walrus allows only one sync-wait per instruction. Hoist
    extra semaphore waits onto single-wait NoOps inserted just before the
    instruction on the same engine stream (same per-engine program order,
    identical blocking semantics)."""
    import bass_rust

    n_split = 0
    for f in nc.m.functions:
        for bb in f.blocks:
            insts = bb.instructions
            i = 0
            while i < len(insts):
                ins = insts[i]
                si = ins.sync_info
                if si is not None:
                    waits = list(si.on_wait)
                    sem_waits = [w for w in waits if w.sync_type == "semaphore"]
                    other = [w for w in waits if w.sync_type != "semaphore"]
                    keep = 1 if not other else 0
                    if len(waits) > 1 and len(sem_waits) > keep:
                        hoist = sem_waits[: len(sem_waits) - keep]
                        kept = sem_waits[len(sem_waits) - keep:]
                        si.on_wait = other + kept
                        for w in hoist:
                            noop = mybir.InstNoOp(
                                name=f"wsplit-{_wsplit_counter[0]}", ins=[], outs=[]
                            )
                            _wsplit_counter[0] += 1
                            noop.engine = ins.engine
                            noop.sync_info = bass_rust.SyncInfo(
                                on_wait=[w], on_update=[]
                            )
                            insts.insert(i, noop)
                            i += 1
                            n_split += 1
                i += 1
    return n_split


def _hoist_to_preamble(nc, names):
    """Move the named (wait-free) instructions from the body block to the
    preamble block, before the startup barrier, so their DMA transfers
    overlap the fixed kernel-start overhead."""
    main_bb = nc.m.functions[0].blocks[0]
    moved = []
    for f in nc.m.functions:
        for bb in f.blocks:
            if bb is main_bb:
                continue
            insts = bb.instructions
            i = 0
            while i < len(insts):
                if insts[i].name in names:
                    moved.append(insts.pop(i))
                else:
                    i += 1
    # insert after the leading Call/RegisterMove run
    pos = 0
    mi = main_bb.instructions
    while pos < len(mi) and mi[pos].opcode in ("Call", "RegisterMove"):
        pos += 1
    for j, ins in enumerate(moved):
        mi.insert(pos + j, ins)
    return len(moved)


def _build():
    f32, bf16 = mybir.dt.float32, mybir.dt.bfloat16
    i16 = mybir.dt.int16
    op = mybir.AluOpType

    nc = bass.Bass()
    blob = nc.declare_dram_parameter("blob", [128, BLOB_COLS], i16, isOutput=False)
    out = nc.declare_dram_parameter("out", [TOK, D], f32, isOutput=True)

    hoist_names = []
    with _LeanTailTileContext(nc) as tc:
        with (
            tc.tile_pool(name="const", bufs=1) as cpool,
            tc.tile_pool(name="outp", bufs=4) as opool,
            tc.tile_pool(name="psum", bufs=1, space="PSUM") as ppool,
        ):
            sb = cpool.tile([128, BLOB_COLS], i16)
            bits_i = cpool.tile([128, TOK], i16)
            bitsT = cpool.tile([128, TOK], bf16)

            wstack = sb[:, 0:W_COLS].bitcast(bf16)
            mks = sb[:, MASK_COL : MASK_COL + 1]

            # input DMAs (hoisted to the preamble by name below)
            d0 = nc.scalar.dma_start(
                sb[:, 0 : X_OFF + CHUNK0], blob[:, 0 : X_OFF + CHUNK0]
            )
            d1 = nc.scalar.dma_start(
                sb[:, X_OFF + CHUNK0 :], blob[:, X_OFF + CHUNK0 :]
            )
            hoist_names = [d0.ins.name, d1.ins.name]

            # bits: (x & mask) != 0 -> bf16 (two DVE ops per chunk)
            off = 0
            for cl in CHUNKS:
                xsl = sb[:, X_OFF + off : X_OFF + off + cl]
                nc.vector.tensor_scalar(
                    bits_i[:, off : off + cl], xsl, mks, None, op.bitwise_and
                )
                nc.vector.tensor_scalar(
                    bitsT[:, off : off + cl],
                    bits_i[:, off : off + cl],
                    0,
                    None,
                    op.not_equal,
                )
                off += cl

            # main loop: 8 supertiles of 512 tokens
            for s in range(NST):
                ob = opool.tile([TILE, NG * D], f32)
                for g in range(NG):
                    t0 = (s * NG + g) * TILE
                    for h in range(2):
                        pt = ppool.tile([TILE, 512], f32, tag=f"p{g}{h}")
                        nc.tensor.matmul(
                            pt[:],
                            bitsT[32 * g : 32 * g + GK, t0 : t0 + TILE],
                            wstack[32 * g : 32 * g + GK, 512 * h : 512 * (h + 1)],
                            start=True,
                            stop=True,
                            tile_position=(32 * g, 0),
                        )
                        dst = ob[:, g * D + 512 * h : g * D + 512 * (h + 1)]
                        if (g + h) % 2 == 0:
                            nc.vector.tensor_copy(dst, pt[:])
                        else:
                            nc.scalar.copy(dst, pt[:])
                if s == 0:
                    # start the output stream ASAP: four 512 KiB DMAs, one
                    # per token tile, issued as soon as each pair of copies
                    # lands
                    for g in range(NG):
                        t0 = (s * NG + g) * TILE
                        eng = nc.sync if g % 2 == 0 else nc.scalar
                        eng.dma_start(
                            out[t0 : t0 + TILE, :],
                            ob[:, g * D : (g + 1) * D],
                        )
                else:
                    # one 2 MiB DMA per supertile; alternate HWDGE rings
                    dram_view = out[s * ST : (s + 1) * ST, :].rearrange(
                        "(g p) d -> p g d", p=TILE
                    )
                    eng = nc.sync if s % 2 == 0 else nc.scalar
                    eng.dma_start(
                        dram_view, ob[:].rearrange("p (g d) -> p g d", g=NG)
                    )

    _hoist_to_preamble(nc, set(hoist_names))
    _split_multi_waits(nc)
    return nc


_nc_cache = None


def _make_blob(xf_shard, weight):
    """Host-staged per-core input blob [128, BLOB_COLS] int16."""
    blob = np.zeros((128, BLOB_COLS), np.int16)
    w = np.asarray(weight, dtype=np.float32)
    hi = w.astype(ml_dtypes.bfloat16)
    lo = (w - hi.astype(np.float32)).astype(ml_dtypes.bfloat16)
    hi16 = hi.view(np.int16)
    lo16 = lo.view(np.int16)
    for g in range(NG):
        blob[32 * g : 32 * g + NB, 0:W_COLS] = hi16
        blob[32 * g + NB : 32 * g + GK, 0:W_COLS] = lo16
    for p in range(128):
        r = p % 32
        if r < GK:
            blob[p, MASK_COL] = 1 << (r % NB)
    blob[:, X_OFF:] = xf_shard[None, :]
    return blob


def kernel(x, weight):
    global _nc_cache, LAST_RESULTS
    if _nc_cache is None:
        _nc_cache = _build()
    nc = _nc_cache

    # x values are < 8192 so they fit int16 exactly
    xf = np.asarray(x, dtype=np.int32).reshape(-1).astype(np.int16)
    in_maps = [
        {"blob": _make_blob(xf[c * TOK : (c + 1) * TOK], weight)}
        for c in range(N_CORES)
    ]
    res = run_bass_kernel_spmd(nc, in_maps, list(range(N_CORES)), trace=TRACE)
    LAST_RESULTS = res
    out = np.concatenate([r["out"] for r in res.results], axis=0)
    return out.reshape(B, S, D)
